# revision 17
# baseline (speedup 1.0000x reference)
"""Hexagonal conv2d (HConv2D) Trainium2 kernel.

Math (verified vs the jax reference to 2.5e-7):
  out[n, 2i,   w, f] = relu(b + a[2i] + bb[2i+1] + c[2i+2])        (w-aligned)
  out[n, 2i+1, w, f] = relu(b + a'[2i+1][w] + c[2i+2][w-1])
with per-input-row 1D convs over Cin=128 -> F=256:
  a[r][w]  = k01.x[r,w]   + k02.x[r,w+1]
  a'[r][w] = k01.x[r,w-1] + k02.x[r,w]
  bb[r][w] = k10.x[r,w-1] + k11.x[r,w] + k12.x[r,w+1]
  c[r][w]  = k21.x[r,w]   + k22.x[r,w+1]
where krc = kernel[r, c] : [Cin, F].  KEY: c[2i+2] is shared between the
even and odd output rows (odd reads it shifted by one column), so it is
computed ONCE on the PE and added into both outputs on the Vector
engine: 9 big tap-matmuls per output-row-pair instead of 11 (18% PE cut).
The odd w=0 seam column (c[-1] = k22.x[0]) is one tiny N=4 matmul.

Distribution: data-parallel over batch (16 -> 8 cores x 2 images). Host
transposes x to [n, c, h, w] (c on partitions = contraction dim), pads
h/w with zeros, casts to bf16. PSUM tiles are [Fchunk=128, 4 rows, 128 w]
(one bank, N=512). Bias+ReLU on ScalarE, DMA out to [n, parity, f, h/2, w]
planes, host reassembles NHWC fp32.
"""

import numpy as np
import ml_dtypes

import concourse.bacc as bacc
import concourse.bass as bass
import concourse.mybir as mybir
import concourse.tile as tile
from concourse.bass_utils import run_bass_kernel_spmd

N_CORES = 8
NPC = 2            # images per core
H = W = 128
C = 128            # input channels
F = 256            # filters
HP, WP = H + 1, W + 2
HB = 4             # out-row-pairs per psum tile (4 pairs -> N=512)
NHB = (H // 2) // HB

# tap weight order: kernel[r][c] for these (r, c)
TAP_RC = [(0, 1), (0, 2), (1, 0), (1, 1), (1, 2), (2, 1), (2, 2)]

BF16 = mybir.dt.bfloat16
F32 = mybir.dt.float32


def _build():
    nc = bacc.Bacc(
        "TRN2", target_bir_lowering=False, debug=False, num_devices=N_CORES
    )
    xt = nc.dram_tensor("xt", (NPC, C, HP, WP), BF16, kind="ExternalInput").ap()
    wt = nc.dram_tensor("wt", (C, 7, F), BF16, kind="ExternalInput").ap()
    bs = nc.dram_tensor("bs", (F // 2, 2), F32, kind="ExternalInput").ap()
    ot = nc.dram_tensor(
        "ot", (NPC, 2, F, H // 2, W), F32, kind="ExternalOutput"
    ).ap()

    with tile.TileContext(nc) as tc:
        with (
            tc.tile_pool(name="const", bufs=1) as const,
            tc.tile_pool(name="xpool", bufs=1) as xpool,
            tc.tile_pool(name="psum", bufs=2, space="PSUM") as psum,
            tc.tile_pool(name="osb", bufs=6) as osb,
        ):
            xs = [
                xpool.tile([C, HP, WP], BF16, name=f"xs{n}", tag=f"xs{n}")
                for n in range(NPC)
            ]
            # first chunk small so the first matmul group starts ASAP
            nc.sync.dma_start(out=xs[0][:, 0:9, :], in_=xt[0, :, 0:9, :])
            w_sb = const.tile([C, 7, F], BF16, name="w_sb")
            nc.sync.dma_start(out=w_sb[:], in_=wt[:])
            b_sb = const.tile([F // 2, 2], F32, name="b_sb")
            nc.sync.dma_start(out=b_sb[:], in_=bs[:])
            for n in range(NPC):
                for h0 in ([9, 41, 73, 105] if n == 0 else [0, 33, 66, 99]):
                    h1 = min(h0 + (32 if n == 0 else 33), HP)
                    nc.sync.dma_start(
                        out=xs[n][:, h0:h1, :], in_=xt[n, :, h0:h1, :]
                    )

            # Warm the PE HAM clock-gate during the input-DMA wait: dummy
            # matmuls on a zeroed scratch tile fill the otherwise-idle
            # window between engine preamble and first data, so the real
            # matmuls start at 2.4GHz instead of the cold 1.2GHz.
            warm_sb = const.tile([128, 512], BF16, name="warm_sb")
            nc.vector.memset(warm_sb[:], 0.0)
            ps_w = psum.tile([128, 512], F32, name="ps_w", tag="ps_c")
            for _ in range(6):
                nc.tensor.matmul(
                    ps_w[:], warm_sb[:, 0:128], warm_sb[:],
                    start=True, stop=True)

            add = mybir.AluOpType.add
            for n in range(NPC):
                for hb in range(NHB):
                    r0 = 2 * HB * hb
                    rE = slice(r0, r0 + 2 * HB - 1, 2)        # rows 2i
                    rO = slice(r0 + 1, r0 + 2 * HB, 2)        # rows 2i+1
                    rC = slice(r0 + 2, r0 + 2 * HB + 1, 2)    # rows 2i+2
                    for fj in range(2):
                        fsl = slice(fj * 128, (fj + 1) * 128)

                        # Three PSUM accumulation groups, matmuls interleaved
                        # weight-major so identical stationary weights are
                        # back-to-back and ps_c finishes early (DVE pipeline).
                        # c[2i+2][w] = k21.x[w] + k22.x[w+1], w = 0..127
                        ps_c = psum.tile([128, HB, W], F32, name="ps_c", tag="ps_c")
                        # even: a[2i] + bb[2i+1]
                        ps_e = psum.tile(
                            [128, HB, W], F32, name="ps_e", tag="ps_e", bufs=3
                        )
                        # odd: a'[2i+1]; w=0 seam adds c[-1] = k22.x[0]
                        ps_o = psum.tile(
                            [128, HB, W], F32, name="ps_o", tag="ps_o", bufs=3
                        )
                        mm = nc.tensor.matmul
                        mm(ps_c[:], w_sb[:, 5, fsl], xs[n][:, rC, 1:129],
                           start=True, stop=False)
                        mm(ps_c[:], w_sb[:, 6, fsl], xs[n][:, rC, 2:130],
                           start=False, stop=True)
                        mm(ps_o[:, :, 0:1], w_sb[:, 6, fsl], xs[n][:, rC, 1:2],
                           start=True, stop=False)
                        mm(ps_o[:], w_sb[:, 0, fsl], xs[n][:, rO, 0:128],
                           start=False, stop=False)
                        mm(ps_o[:], w_sb[:, 1, fsl], xs[n][:, rO, 1:129],
                           start=False, stop=True)
                        mm(ps_e[:], w_sb[:, 0, fsl], xs[n][:, rE, 1:129],
                           start=True, stop=False)
                        mm(ps_e[:], w_sb[:, 1, fsl], xs[n][:, rE, 2:130],
                           start=False, stop=False)
                        mm(ps_e[:], w_sb[:, 2, fsl], xs[n][:, rO, 0:128],
                           start=False, stop=False)
                        mm(ps_e[:], w_sb[:, 3, fsl], xs[n][:, rO, 1:129],
                           start=False, stop=False)
                        mm(ps_e[:], w_sb[:, 4, fsl], xs[n][:, rO, 2:130],
                           start=False, stop=True)

                        # DVE cannot read two PSUM operands in one op:
                        # stage c in SBUF, then add it into both parities.
                        # Alternate the copy engine to balance DVE vs ACT load.
                        c_sb = osb.tile([128, HB, W], F32, name="c_sb", tag="c_sb")
                        if (hb + fj) % 2 == 0:
                            nc.vector.tensor_copy(c_sb[:], ps_c[:])
                        else:
                            nc.scalar.copy(c_sb[:], ps_c[:])
                        ob_e = osb.tile([128, HB, W], F32, name="ob_e", tag="ob_e")
                        nc.vector.tensor_tensor(
                            ob_e[:], ps_e[:], c_sb[:], op=add)
                        ob_o = osb.tile([128, HB, W], F32, name="ob_o", tag="ob_o")
                        nc.vector.tensor_tensor(
                            ob_o[:, :, 1:128], ps_o[:, :, 1:128],
                            c_sb[:, :, 0:127], op=add)
                        nc.vector.tensor_copy(ob_o[:, :, 0:1], ps_o[:, :, 0:1])

                        for par, ob in ((0, ob_e), (1, ob_o)):
                            fo = osb.tile(
                                [128, HB, W], F32, name=f"fo{par}", tag=f"fo{par}"
                            )
                            nc.scalar.activation(
                                fo[:], ob[:],
                                mybir.ActivationFunctionType.Relu,
                                bias=b_sb[:, fj : fj + 1],
                            )
                            nc.sync.dma_start(
                                out=ot[n, par, fsl, hb * HB : (hb + 1) * HB, :],
                                in_=fo[:],
                            )
    nc.compile()
    return nc


_NC_CACHE = None


def _get_nc():
    global _NC_CACHE
    if _NC_CACHE is None:
        _NC_CACHE = _build()
    return _NC_CACHE


def _prep_core_inputs(x_shard, wt_host, bs_host):
    xp = np.zeros((NPC, C, HP, WP), dtype=ml_dtypes.bfloat16)
    xp[:, :, :H, 1 : 1 + W] = x_shard.transpose(0, 3, 1, 2)
    return {"xt": xp, "wt": wt_host, "bs": bs_host}


def kernel(x, kernel, bias):
    x = np.asarray(x, dtype=np.float32)
    kernel = np.asarray(kernel, dtype=np.float32)
    bias = np.asarray(bias, dtype=np.float32)

    wt_host = np.stack(
        [kernel[r, c] for (r, c) in TAP_RC], axis=1
    ).astype(ml_dtypes.bfloat16)  # (C, 7, F)
    bs_host = np.ascontiguousarray(
        bias.reshape(2, F // 2).T
    ).astype(np.float32)  # (128, 2): bs[f, j] = bias[j*128+f]

    nc = _get_nc()
    in_maps = [
        _prep_core_inputs(x[i * NPC : (i + 1) * NPC], wt_host, bs_host)
        for i in range(N_CORES)
    ]
    res = run_bass_kernel_spmd(nc, in_maps, list(range(N_CORES)))

    outs = [res.results[i]["ot"] for i in range(N_CORES)]  # (NPC,2,F,H/2,W)
    full = np.concatenate(outs, axis=0)  # (16, 2, F, H/2, W)
    # out[n, h, w, f] with h = 2*h2 + parity
    out = full.transpose(0, 3, 1, 4, 2).reshape(16, H, W, F)
    return np.ascontiguousarray(out)


# revision 18
# speedup vs baseline: 1.0120x; 1.0120x over previous
"""Hexagonal conv2d (HConv2D) Trainium2 kernel.

Math (verified vs the jax reference to 2.5e-7):
  out[n, 2i,   w, f] = relu(b + a[2i] + bb[2i+1] + c[2i+2])        (w-aligned)
  out[n, 2i+1, w, f] = relu(b + a'[2i+1][w] + c[2i+2][w-1])
with per-input-row 1D convs over Cin=128 -> F=256:
  a[r][w]  = k01.x[r,w]   + k02.x[r,w+1]
  a'[r][w] = k01.x[r,w-1] + k02.x[r,w]
  bb[r][w] = k10.x[r,w-1] + k11.x[r,w] + k12.x[r,w+1]
  c[r][w]  = k21.x[r,w]   + k22.x[r,w+1]
where krc = kernel[r, c] : [Cin, F].  KEY: c[2i+2] is shared between the
even and odd output rows (odd reads it shifted by one column), so it is
computed ONCE on the PE and added into both outputs on the Vector
engine: 9 big tap-matmuls per output-row-pair instead of 11 (18% PE cut).
The odd w=0 seam column (c[-1] = k22.x[0]) is one tiny N=4 matmul.

Distribution: data-parallel over batch (16 -> 8 cores x 2 images). Host
transposes x to [n, c, h, w] (c on partitions = contraction dim), pads
h/w with zeros, casts to bf16. PSUM tiles are [Fchunk=128, 4 rows, 128 w]
(one bank, N=512). Bias+ReLU on ScalarE, DMA out to [n, parity, f, h/2, w]
planes, host reassembles NHWC fp32.
"""

import numpy as np
import ml_dtypes

import concourse.bacc as bacc
import concourse.bass as bass
import concourse.mybir as mybir
import concourse.tile as tile
from concourse.bass_utils import run_bass_kernel_spmd

N_CORES = 8
NPC = 2            # images per core
H = W = 128
C = 128            # input channels
F = 256            # filters
HP, WP = H + 1, W + 2
HB = 4             # out-row-pairs per psum tile (4 pairs -> N=512)
NHB = (H // 2) // HB

# tap weight order: kernel[r][c] for these (r, c)
TAP_RC = [(0, 1), (0, 2), (1, 0), (1, 1), (1, 2), (2, 1), (2, 2)]

BF16 = mybir.dt.bfloat16
F32 = mybir.dt.float32


def _build():
    nc = bacc.Bacc(
        "TRN2", target_bir_lowering=False, debug=False, num_devices=N_CORES
    )
    xt = nc.dram_tensor("xt", (NPC, C, HP, WP), BF16, kind="ExternalInput").ap()
    wt = nc.dram_tensor("wt", (C, 7, F), BF16, kind="ExternalInput").ap()
    bs = nc.dram_tensor("bs", (F // 2, 2), F32, kind="ExternalInput").ap()
    ot = nc.dram_tensor(
        "ot", (NPC, 2, F, H // 2, W), F32, kind="ExternalOutput"
    ).ap()

    with tile.TileContext(nc) as tc:
        with (
            tc.tile_pool(name="const", bufs=1) as const,
            tc.tile_pool(name="xpool", bufs=1) as xpool,
            tc.tile_pool(name="psum", bufs=2, space="PSUM") as psum,
            tc.tile_pool(name="osb", bufs=6) as osb,
        ):
            xs = [
                xpool.tile([C, HP, WP], BF16, name=f"xs{n}", tag=f"xs{n}")
                for n in range(NPC)
            ]
            # first chunk small so the first matmul group starts ASAP
            nc.sync.dma_start(out=xs[0][:, 0:9, :], in_=xt[0, :, 0:9, :])
            w_sb = const.tile([C, 7, F], BF16, name="w_sb")
            nc.sync.dma_start(out=w_sb[:], in_=wt[:])
            b_sb = const.tile([F // 2, 2], F32, name="b_sb")
            nc.sync.dma_start(out=b_sb[:], in_=bs[:])
            for n in range(NPC):
                for h0 in ([9, 41, 73, 105] if n == 0 else [0, 33, 66, 99]):
                    h1 = min(h0 + (32 if n == 0 else 33), HP)
                    nc.sync.dma_start(
                        out=xs[n][:, h0:h1, :], in_=xt[n, :, h0:h1, :]
                    )

            # Warm the PE HAM clock-gate during the input-DMA wait: dummy
            # matmuls on a zeroed scratch tile fill the otherwise-idle
            # window between engine preamble and first data, so the real
            # matmuls start at 2.4GHz instead of the cold 1.2GHz.
            warm_sb = const.tile([128, 512], BF16, name="warm_sb")
            nc.vector.memset(warm_sb[:], 0.0)
            ps_w = psum.tile([128, 512], F32, name="ps_w", tag="ps_c")
            for _ in range(10):
                nc.tensor.matmul(
                    ps_w[:], warm_sb[:, 0:128], warm_sb[:],
                    start=True, stop=True)

            add = mybir.AluOpType.add
            for n in range(NPC):
                for hb in range(NHB):
                    r0 = 2 * HB * hb
                    rE = slice(r0, r0 + 2 * HB - 1, 2)        # rows 2i
                    rO = slice(r0 + 1, r0 + 2 * HB, 2)        # rows 2i+1
                    rC = slice(r0 + 2, r0 + 2 * HB + 1, 2)    # rows 2i+2
                    for fj in range(2):
                        fsl = slice(fj * 128, (fj + 1) * 128)

                        # Three PSUM accumulation groups, matmuls interleaved
                        # weight-major so identical stationary weights are
                        # back-to-back and ps_c finishes early (DVE pipeline).
                        # c[2i+2][w] = k21.x[w] + k22.x[w+1], w = 0..127
                        ps_c = psum.tile([128, HB, W], F32, name="ps_c", tag="ps_c")
                        # even: a[2i] + bb[2i+1]
                        ps_e = psum.tile(
                            [128, HB, W], F32, name="ps_e", tag="ps_e", bufs=3
                        )
                        # odd: a'[2i+1]; w=0 seam adds c[-1] = k22.x[0]
                        ps_o = psum.tile(
                            [128, HB, W], F32, name="ps_o", tag="ps_o", bufs=3
                        )
                        mm = nc.tensor.matmul
                        mm(ps_c[:], w_sb[:, 5, fsl], xs[n][:, rC, 1:129],
                           start=True, stop=False)
                        mm(ps_c[:], w_sb[:, 6, fsl], xs[n][:, rC, 2:130],
                           start=False, stop=True)
                        mm(ps_o[:, :, 0:1], w_sb[:, 6, fsl], xs[n][:, rC, 1:2],
                           start=True, stop=False)
                        mm(ps_o[:], w_sb[:, 0, fsl], xs[n][:, rO, 0:128],
                           start=False, stop=False)
                        mm(ps_o[:], w_sb[:, 1, fsl], xs[n][:, rO, 1:129],
                           start=False, stop=True)
                        mm(ps_e[:], w_sb[:, 0, fsl], xs[n][:, rE, 1:129],
                           start=True, stop=False)
                        mm(ps_e[:], w_sb[:, 1, fsl], xs[n][:, rE, 2:130],
                           start=False, stop=False)
                        mm(ps_e[:], w_sb[:, 2, fsl], xs[n][:, rO, 0:128],
                           start=False, stop=False)
                        mm(ps_e[:], w_sb[:, 3, fsl], xs[n][:, rO, 1:129],
                           start=False, stop=False)
                        mm(ps_e[:], w_sb[:, 4, fsl], xs[n][:, rO, 2:130],
                           start=False, stop=True)

                        # DVE cannot read two PSUM operands in one op:
                        # stage c in SBUF, then add it into both parities.
                        # Alternate the copy engine to balance DVE vs ACT load.
                        c_sb = osb.tile([128, HB, W], F32, name="c_sb", tag="c_sb")
                        if (hb + fj) % 2 == 0:
                            nc.vector.tensor_copy(c_sb[:], ps_c[:])
                        else:
                            nc.scalar.copy(c_sb[:], ps_c[:])
                        ob_e = osb.tile([128, HB, W], F32, name="ob_e", tag="ob_e")
                        nc.vector.tensor_tensor(
                            ob_e[:], ps_e[:], c_sb[:], op=add)
                        ob_o = osb.tile([128, HB, W], F32, name="ob_o", tag="ob_o")
                        nc.vector.tensor_tensor(
                            ob_o[:, :, 1:128], ps_o[:, :, 1:128],
                            c_sb[:, :, 0:127], op=add)
                        nc.vector.tensor_copy(ob_o[:, :, 0:1], ps_o[:, :, 0:1])

                        for par, ob in ((0, ob_e), (1, ob_o)):
                            fo = osb.tile(
                                [128, HB, W], F32, name=f"fo{par}", tag=f"fo{par}"
                            )
                            nc.scalar.activation(
                                fo[:], ob[:],
                                mybir.ActivationFunctionType.Relu,
                                bias=b_sb[:, fj : fj + 1],
                            )
                            nc.sync.dma_start(
                                out=ot[n, par, fsl, hb * HB : (hb + 1) * HB, :],
                                in_=fo[:],
                            )
    nc.compile()
    return nc


_NC_CACHE = None


def _get_nc():
    global _NC_CACHE
    if _NC_CACHE is None:
        _NC_CACHE = _build()
    return _NC_CACHE


def _prep_core_inputs(x_shard, wt_host, bs_host):
    xp = np.zeros((NPC, C, HP, WP), dtype=ml_dtypes.bfloat16)
    xp[:, :, :H, 1 : 1 + W] = x_shard.transpose(0, 3, 1, 2)
    return {"xt": xp, "wt": wt_host, "bs": bs_host}


def kernel(x, kernel, bias):
    x = np.asarray(x, dtype=np.float32)
    kernel = np.asarray(kernel, dtype=np.float32)
    bias = np.asarray(bias, dtype=np.float32)

    wt_host = np.stack(
        [kernel[r, c] for (r, c) in TAP_RC], axis=1
    ).astype(ml_dtypes.bfloat16)  # (C, 7, F)
    bs_host = np.ascontiguousarray(
        bias.reshape(2, F // 2).T
    ).astype(np.float32)  # (128, 2): bs[f, j] = bias[j*128+f]

    nc = _get_nc()
    in_maps = [
        _prep_core_inputs(x[i * NPC : (i + 1) * NPC], wt_host, bs_host)
        for i in range(N_CORES)
    ]
    res = run_bass_kernel_spmd(nc, in_maps, list(range(N_CORES)))

    outs = [res.results[i]["ot"] for i in range(N_CORES)]  # (NPC,2,F,H/2,W)
    full = np.concatenate(outs, axis=0)  # (16, 2, F, H/2, W)
    # out[n, h, w, f] with h = 2*h2 + parity
    out = full.transpose(0, 3, 1, 4, 2).reshape(16, H, W, F)
    return np.ascontiguousarray(out)
